# revision 33
# baseline (speedup 1.0000x reference)
"""LongcatFlashTopkRouter on 8 Trainium2 NeuronCores.

Math (per token t):
    logits = h_t @ W.T                      # [768]
    s      = softmax(logits)
    c      = s + bias                       # bias-corrected selection scores
    idx    = top12(c)                       # descending, ties -> lower index
    w      = 2.5 * s[idx] / sum(s[idx])

Device-side reformulation (per token, no softmax materialization needed):
    e   = exp(logits)           (no max-subtraction: |logits| < ~9 is safe in fp32)
    se  = sum(e)
    z   = e + se * bias         # z has the SAME ordering as c = e/se + bias
    top-8 of each 256-expert third of z (DVE max/max_index, 6 short scans
    instead of 5 full-width ones -> ~2x less DVE time)
Host epilogue (cheap, vectorized numpy):
    merge 24 candidates -> top-16; e16 = z16 - se * bias[idx16]
    w   = 2.5 * e16[:, :12] / sum(e16[:, :12])   # the 1/se factor cancels
    tokens whose top-13 adjacent z-gaps are inside the noise band, or where
    one third contributed >=8 of the top-13 (a 9th candidate could be
    hidden), are recomputed exactly in fp32 on host.

Sharding: tokens (batch*seq = 32768) split evenly across 8 cores (4096 each);
W and bias replicated. Hidden states are pre-transposed on the host into
[tile, k-partition, k-chunk, token] layout so each 128-token tile's 16
contraction chunks are contiguous SBUF-ready [128, 2048] blocks.

Matmul runs in float16 (full-rate PE mode like bf16/fp32r but half the HBM
traffic of fp32; logit noise ~2e-4 relative, on par with float32r).
fp8 was measured 2.25x faster on the PE but its quantization noise (~2.3e-2
on logits) scrambles the top-12 boundary for most tokens; error-compensated
fp8 splits need >=3 chains and end up slower than one fp16 pass.
"""

import numpy as np

import concourse.bass as bass
import concourse.mybir as mybir
from concourse import bacc
from concourse.tile import TileContext
from concourse.bass_utils import run_bass_kernel_spmd

N_CORES = 8
B, S, H, E = 4, 8192, 2048, 768
TOK = B * S // N_CORES      # 4096 tokens per core
TT = 32                     # token tiles of 128 per core
KC = H // 128               # 16 contraction chunks
TOPK = 12
TOP16 = 16
SCALE = 2.5

F32 = mybir.dt.float32
F32R = mybir.dt.float32r
F16 = mybir.dt.float16
U32 = mybir.dt.uint32
EXP = mybir.ActivationFunctionType.Exp
COPY = mybir.ActivationFunctionType.Copy

PRO_T = 2                   # tiles in the chunk-major warmup


def build_nc(mm_dtype=F16):
    nc = bacc.Bacc()
    ht = nc.dram_tensor("ht", [TT, 128, KC, 128], mm_dtype, kind="ExternalInput")
    wt = nc.dram_tensor("wt", [128, KC, E], mm_dtype, kind="ExternalInput")
    biasb = nc.dram_tensor("biasb", [128, E], F32, kind="ExternalInput")
    # packed per-tile output: [z24 f32 | idx24 u16 (12 f32 slots) | se f32]
    o_pack = nc.dram_tensor("o_pack", [TT, 128, 37], F32, kind="ExternalOutput")

    HK = KC // 2 * 128      # half-tile free size (8 chunks)

    with TileContext(nc) as tc:
        with (
            tc.tile_pool(name="const", bufs=1) as cpool,
            tc.tile_pool(name="hin", bufs=6) as hpool,
            tc.tile_pool(name="mid", bufs=3) as mpool,
            tc.tile_pool(name="small", bufs=6) as spool,
            tc.tile_pool(name="ps", bufs=4, space="PSUM") as ppool,
        ):
            # DMA plan (3 HWDGE queues: sync, scalar, gpsimd).  h0 and h1
            # are split in thirds across all three queues so they land in
            # ~1/3 the single-queue time; then the 16 wt chunks stream
            # round-robin (one queue each), arriving slightly ahead of the
            # 2-tile chunk-major warmup's ~1.3us/chunk consumption rate.
            QS = (nc.gpsimd, nc.sync, nc.scalar)
            h_tiles = {}
            for t in range(PRO_T):
                h_tiles[t] = hpool.tile(
                    [128, KC * 128], mm_dtype, tag="h", name=f"h_p{t}"
                )
            CSPLIT = (0, 6, 11, KC)  # chunk ranges per queue third
            wt_sb = [
                cpool.tile([128, E], mm_dtype, tag=f"wt{c}", name=f"wt_c{c}")
                for c in range(KC)
            ]
            # per-queue order: h0 third, first wt chunk (wt0/1/2 -> needed
            # first by the warmup), h1 third, remaining wt chunks
            for q in range(3):
                lo, hi = CSPLIT[q], CSPLIT[q + 1]
                QS[q].dma_start(out=h_tiles[0][:, lo * 128:hi * 128],
                                in_=ht[0][:, lo:hi])
            for c in range(3):
                QS[c].dma_start(out=wt_sb[c], in_=wt[:, c])
            for q in range(3):
                lo, hi = CSPLIT[q], CSPLIT[q + 1]
                QS[q].dma_start(out=h_tiles[1][:, lo * 128:hi * 128],
                                in_=ht[1][:, lo:hi])
            for c in range(3, KC):
                QS[c % 3].dma_start(out=wt_sb[c], in_=wt[:, c])
            bias_sb = cpool.tile([128, E], F32)
            nc.gpsimd.dma_start(out=bias_sb, in_=biasb[:])

            def mm_tile(h_sb, ps, c):
                lhsT = h_sb[:, c * 128:(c + 1) * 128]
                nc.tensor.matmul(
                    ps[:, 0:512], lhsT, wt_sb[c][:, 0:512],
                    start=(c == 0), stop=(c == KC - 1),
                )
                nc.tensor.matmul(
                    ps[:, 512:E], lhsT, wt_sb[c][:, 512:E],
                    start=(c == 0), stop=(c == KC - 1),
                )

            def post_tile(t, ps):
                # packed result tile: z24(f32) | idx24(u16, 12 f32 slots) |
                # se(f32).  Top-8 of each 256-expert third; the host merges
                # the 24 candidates into the top-16 (a third can hide a true
                # top-12 entry only when it holds >=9 of them, which the
                # host detects and fixes via the at-risk path).
                comb = spool.tile([128, 37], F32, tag="comb")
                se = comb[:, 36:37]
                zt = comb[:, 0:24]                             # [128, 24] f32
                i24 = comb[:, 24:36].bitcast(mybir.dt.uint16)  # [128, 24] u16

                # e = exp(logits), se = rowsum(e) (ScalarE, single pass)
                ez = mpool.tile([128, E], F32, tag="ez")
                nc.scalar.activation(out=ez, in_=ps, func=EXP, accum_out=se)

                # br = bias * se (ScalarE, per-partition scale)
                br = mpool.tile([128, E], F32, tag="br")
                nc.scalar.activation(out=br, in_=bias_sb, func=COPY, scale=se)

                # z = e + br on the otherwise-idle GpSimd, split per third so
                # the DVE's first scan starts ~1.2us earlier in the chain
                z = mpool.tile([128, E], F32, tag="z")
                for j in range(3):
                    lo, hi = j * 256, (j + 1) * 256
                    nc.gpsimd.tensor_add(z[:, lo:hi], ez[:, lo:hi], br[:, lo:hi])
                    nc.vector.max(zt[:, j * 8:(j + 1) * 8], z[:, lo:hi])
                    nc.vector.max_index(i24[:, j * 8:(j + 1) * 8],
                                        zt[:, j * 8:(j + 1) * 8], z[:, lo:hi])

                # pack DMA rides the sync queue: its wait on the DVE must not
                # block the scalar queue's EXP/COPY for the next tiles
                nc.sync.dma_start(out=o_pack[t], in_=comb)

            # chunk-major warmup over the first PRO_T tiles so the PE starts
            # as soon as h0 + wt0 land instead of waiting for the whole wt
            ps_pro = [
                ppool.tile([128, E], F32, tag="ps", name=f"ps_pro{i}")
                for i in range(PRO_T)
            ]
            for c in range(KC):
                for t in range(PRO_T):
                    mm_tile(h_tiles[t], ps_pro[t], c)
            for t in range(PRO_T):
                post_tile(t, ps_pro[t])

            # steady state: tile-major; h DMAs rotate across the 3 queues
            for t in range(PRO_T, TT):
                h_sb = hpool.tile([128, KC * 128], mm_dtype, tag="h")
                QS[t % 3].dma_start(out=h_sb, in_=ht[t])
                ps = ppool.tile([128, E], F32, tag="ps")
                for c in range(KC):
                    mm_tile(h_sb, ps, c)
                post_tile(t, ps)
    nc.finalize()
    return nc


def _np_mm_dtype(mm_dtype):
    return np.float16 if mm_dtype == F16 else np.float32


def _prep_inputs(h, W_, b, mm_dtype=F16):
    npdt = _np_mm_dtype(mm_dtype)
    # [k_in_chunk(p), chunk(c), expert(e)]: wtprep[p, c, e] = W[e, c*128 + p]
    wtprep = np.ascontiguousarray(
        W_.T.reshape(KC, 128, E).transpose(1, 0, 2).astype(npdt)
    )
    biasb = np.ascontiguousarray(np.broadcast_to(b, (128, E)))
    in_maps = []
    for core in range(N_CORES):
        hc = h[core * TOK:(core + 1) * TOK]
        # [tile, token_in_tile(j), chunk(c), k_in_chunk(p)] -> [tile, p, c, j]
        h4 = hc.reshape(TT, 128, KC, 128)
        htp = np.ascontiguousarray(h4.transpose(0, 3, 2, 1).astype(npdt))
        in_maps.append({"ht": htp, "wt": wtprep, "biasb": biasb})
    return in_maps


RISK_TAU = 1e-3  # local relative z-gap below which noise could flip order
_DBG = {}


def _epilogue(results, b, h_flat, W):
    idx_list, w_list, risk_list = [], [], []
    for r in results:
        pack = np.ascontiguousarray(r["o_pack"].reshape(-1, 37))
        z24 = pack[:, 0:24]                                       # [N, 24]
        i24 = pack[:, 24:36].view(np.uint16).astype(np.int32)     # local idx
        i24 = i24 + (np.arange(3, dtype=np.int32) * 256).repeat(8)[None, :]
        se = pack[:, 36:37]
        # merge the 3 per-third top-8 lists into a global top-16
        order = np.argsort(-z24, axis=-1, kind="stable")[:, :TOP16]
        z16 = np.take_along_axis(z24, order, axis=-1)
        idx16 = np.take_along_axis(i24, order, axis=-1)
        e16 = (z16 - se * b[idx16]).astype(np.float32)
        e12 = e16[:, :TOPK]
        denom = e12.sum(axis=-1, keepdims=True, dtype=np.float32) + np.float32(1e-20) * se
        w_list.append((np.float32(SCALE) * e12 / denom).astype(np.float32))
        idx_list.append(idx16[:, :TOPK].astype(np.int32))
        # flag tokens whose adjacent top-13 gaps are inside the noise band
        # (relative to the local z, not z1), or where one third contributed
        # >= 8 of the merged top-13 (its 9th candidate could be hidden)
        gaps = (z16[:, :TOPK + 1] - z16[:, 1:TOPK + 2]) / np.abs(z16[:, :TOPK + 1])
        third = order[:, :TOPK + 1] // 8
        crowd = (
            (third == 0).sum(-1) >= 8
        ) | ((third == 1).sum(-1) >= 8) | ((third == 2).sum(-1) >= 8)
        risk_list.append((gaps.min(axis=-1) < RISK_TAU) | crowd)
    topk_idx = np.concatenate(idx_list, axis=0)
    topk_w = np.concatenate(w_list, axis=0)

    # fp32-exact host recompute for at-risk tokens (mimics the reference op
    # sequence exactly in float32)
    risk = np.concatenate(risk_list, axis=0)
    _DBG["risk_frac"] = float(risk.mean())
    ridx = np.nonzero(risk)[0]
    if ridx.size:
        lg = h_flat[ridx] @ W.T.astype(np.float32)
        mx = lg.max(axis=-1, keepdims=True)
        ex = np.exp(lg - mx)
        s = ex / ex.sum(axis=-1, keepdims=True, dtype=np.float32)
        c = s + b
        ii = np.argsort(-c, axis=-1, kind="stable")[:, :TOPK]
        ww = np.take_along_axis(s, ii, axis=-1)
        ww = ww / (ww.sum(axis=-1, keepdims=True, dtype=np.float32) + np.float32(1e-20))
        topk_idx[ridx] = ii.astype(np.int32)
        topk_w[ridx] = (np.float32(SCALE) * ww).astype(np.float32)

    topk_idx = topk_idx.reshape(B, S, TOPK)
    topk_w = topk_w.reshape(B, S, TOPK).astype(np.float32)
    return topk_idx, topk_w


_NC_CACHE = {}


def run(hidden_states, W, e_score_correction_bias, trace=False, mm_dtype=F16):
    key = (str(mm_dtype),)
    if key not in _NC_CACHE:
        _NC_CACHE[key] = build_nc(mm_dtype)
    nc = _NC_CACHE[key]
    h = np.ascontiguousarray(np.asarray(hidden_states, dtype=np.float32)).reshape(-1, H)
    W_ = np.ascontiguousarray(np.asarray(W, dtype=np.float32))
    b = np.ascontiguousarray(np.asarray(e_score_correction_bias, dtype=np.float32))
    in_maps = _prep_inputs(h, W_, b, mm_dtype)
    res = run_bass_kernel_spmd(nc, in_maps, core_ids=list(range(N_CORES)), trace=trace)
    out = _epilogue(res.results, b, h, W_)
    if _DBG:
        print(f"risk fraction: {_DBG.get('risk_frac', -1):.4f}")
    return out, res


def kernel(hidden_states, W, e_score_correction_bias):
    out, _ = run(hidden_states, W, e_score_correction_bias, trace=False)
    return out


# revision 34
# speedup vs baseline: 1.0016x; 1.0016x over previous
"""LongcatFlashTopkRouter on 8 Trainium2 NeuronCores.

Math (per token t):
    logits = h_t @ W.T                      # [768]
    s      = softmax(logits)
    c      = s + bias                       # bias-corrected selection scores
    idx    = top12(c)                       # descending, ties -> lower index
    w      = 2.5 * s[idx] / sum(s[idx])

Device-side reformulation (per token, no softmax materialization needed):
    e   = exp(logits)           (no max-subtraction: |logits| < ~9 is safe in fp32)
    se  = sum(e)
    z   = e + se * bias         # z has the SAME ordering as c = e/se + bias
    top-8 of each 256-expert third of z (DVE max/max_index, 6 short scans
    instead of 5 full-width ones -> ~2x less DVE time)
Host epilogue (cheap, vectorized numpy):
    merge 24 candidates -> top-16; e16 = z16 - se * bias[idx16]
    w   = 2.5 * e16[:, :12] / sum(e16[:, :12])   # the 1/se factor cancels
    tokens whose top-13 adjacent z-gaps are inside the noise band, or where
    one third contributed >=8 of the top-13 (a 9th candidate could be
    hidden), are recomputed exactly in fp32 on host.

Sharding: tokens (batch*seq = 32768) split evenly across 8 cores (4096 each);
W and bias replicated. Hidden states are pre-transposed on the host into
[tile, k-partition, k-chunk, token] layout so each 128-token tile's 16
contraction chunks are contiguous SBUF-ready [128, 2048] blocks.

Matmul runs in float16 (full-rate PE mode like bf16/fp32r but half the HBM
traffic of fp32; logit noise ~2e-4 relative, on par with float32r).
fp8 was measured 2.25x faster on the PE but its quantization noise (~2.3e-2
on logits) scrambles the top-12 boundary for most tokens; error-compensated
fp8 splits need >=3 chains and end up slower than one fp16 pass.
"""

import numpy as np

import concourse.bass as bass
import concourse.mybir as mybir
from concourse import bacc
from concourse.tile import TileContext
from concourse.bass_utils import run_bass_kernel_spmd

N_CORES = 8
B, S, H, E = 4, 8192, 2048, 768
TOK = B * S // N_CORES      # 4096 tokens per core
TT = 32                     # token tiles of 128 per core
KC = H // 128               # 16 contraction chunks
TOPK = 12
TOP16 = 16
SCALE = 2.5

F32 = mybir.dt.float32
F32R = mybir.dt.float32r
F16 = mybir.dt.float16
U32 = mybir.dt.uint32
EXP = mybir.ActivationFunctionType.Exp
COPY = mybir.ActivationFunctionType.Copy

PRO_T = 2                   # tiles in the chunk-major warmup


def build_nc(mm_dtype=F16):
    nc = bacc.Bacc()
    ht = nc.dram_tensor("ht", [TT, 128, KC, 128], mm_dtype, kind="ExternalInput")
    wt = nc.dram_tensor("wt", [128, KC, E], mm_dtype, kind="ExternalInput")
    biasb = nc.dram_tensor("biasb", [128, E], F32, kind="ExternalInput")
    # packed per-tile output: [z24 f32 | idx24 u16 (12 f32 slots) | se f32]
    o_pack = nc.dram_tensor("o_pack", [TT, 128, 37], F32, kind="ExternalOutput")

    HK = KC // 2 * 128      # half-tile free size (8 chunks)

    with TileContext(nc) as tc:
        with (
            tc.tile_pool(name="const", bufs=1) as cpool,
            tc.tile_pool(name="hin", bufs=6) as hpool,
            tc.tile_pool(name="mid", bufs=3) as mpool,
            tc.tile_pool(name="small", bufs=6) as spool,
            tc.tile_pool(name="ps", bufs=4, space="PSUM") as ppool,
        ):
            # DMA plan (3 HWDGE queues: sync, scalar, gpsimd).  h0 and h1
            # are split in thirds across all three queues so they land in
            # ~1/3 the single-queue time; then the 16 wt chunks stream
            # round-robin (one queue each), arriving slightly ahead of the
            # 2-tile chunk-major warmup's ~1.3us/chunk consumption rate.
            QS = (nc.gpsimd, nc.sync, nc.scalar)
            h_tiles = {}
            for t in range(PRO_T):
                h_tiles[t] = hpool.tile(
                    [128, KC * 128], mm_dtype, tag="h", name=f"h_p{t}"
                )
            CSPLIT = (0, 6, 11, KC)  # chunk ranges per queue third
            wt_sb = [
                cpool.tile([128, E], mm_dtype, tag=f"wt{c}", name=f"wt_c{c}")
                for c in range(KC)
            ]
            # per-queue order: h0 third, first wt chunk (wt0/1/2 -> needed
            # first by the warmup), h1 third, remaining wt chunks
            for q in range(3):
                lo, hi = CSPLIT[q], CSPLIT[q + 1]
                QS[q].dma_start(out=h_tiles[0][:, lo * 128:hi * 128],
                                in_=ht[0][:, lo:hi])
            for c in range(3):
                QS[c].dma_start(out=wt_sb[c], in_=wt[:, c])
            for q in range(3):
                lo, hi = CSPLIT[q], CSPLIT[q + 1]
                QS[q].dma_start(out=h_tiles[1][:, lo * 128:hi * 128],
                                in_=ht[1][:, lo:hi])
            for c in range(3, KC):
                QS[c % 3].dma_start(out=wt_sb[c], in_=wt[:, c])
            bias_sb = cpool.tile([128, E], F32)
            nc.gpsimd.dma_start(out=bias_sb, in_=biasb[:])

            def mm_tile(h_sb, ps, c):
                lhsT = h_sb[:, c * 128:(c + 1) * 128]
                nc.tensor.matmul(
                    ps[:, 0:512], lhsT, wt_sb[c][:, 0:512],
                    start=(c == 0), stop=(c == KC - 1),
                )
                nc.tensor.matmul(
                    ps[:, 512:E], lhsT, wt_sb[c][:, 512:E],
                    start=(c == 0), stop=(c == KC - 1),
                )

            def post_tile(t, ps):
                # packed result tile: z24(f32) | idx24(u16, 12 f32 slots) |
                # se(f32).  Top-8 of each 256-expert third; the host merges
                # the 24 candidates into the top-16 (a third can hide a true
                # top-12 entry only when it holds >=9 of them, which the
                # host detects and fixes via the at-risk path).
                comb = spool.tile([128, 37], F32, tag="comb")
                se = comb[:, 36:37]
                zt = comb[:, 0:24]                             # [128, 24] f32
                i24 = comb[:, 24:36].bitcast(mybir.dt.uint16)  # [128, 24] u16

                # e = exp(logits), se = rowsum(e) (ScalarE, single pass)
                ez = mpool.tile([128, E], F32, tag="ez")
                nc.scalar.activation(out=ez, in_=ps, func=EXP, accum_out=se)

                # br = bias * se (ScalarE, per-partition scale)
                br = mpool.tile([128, E], F32, tag="br")
                nc.scalar.activation(out=br, in_=bias_sb, func=COPY, scale=se)

                # z = e + br on the otherwise-idle GpSimd
                z = mpool.tile([128, E], F32, tag="z")
                nc.gpsimd.tensor_add(z, ez, br)

                # top-8 of each third (values + local indices), descending
                for j in range(3):
                    sl = z[:, j * 256:(j + 1) * 256]
                    nc.vector.max(zt[:, j * 8:(j + 1) * 8], sl)
                    nc.vector.max_index(i24[:, j * 8:(j + 1) * 8],
                                        zt[:, j * 8:(j + 1) * 8], sl)

                # pack DMA rides the sync queue: its wait on the DVE must not
                # block the scalar queue's EXP/COPY for the next tiles
                nc.sync.dma_start(out=o_pack[t], in_=comb)

            # chunk-major warmup over the first PRO_T tiles so the PE starts
            # as soon as h0 + wt0 land instead of waiting for the whole wt
            ps_pro = [
                ppool.tile([128, E], F32, tag="ps", name=f"ps_pro{i}")
                for i in range(PRO_T)
            ]
            for c in range(KC):
                for t in range(PRO_T):
                    mm_tile(h_tiles[t], ps_pro[t], c)
            for t in range(PRO_T):
                post_tile(t, ps_pro[t])

            # steady state: tile-major; h DMAs rotate across the 3 queues
            for t in range(PRO_T, TT):
                h_sb = hpool.tile([128, KC * 128], mm_dtype, tag="h")
                QS[t % 3].dma_start(out=h_sb, in_=ht[t])
                ps = ppool.tile([128, E], F32, tag="ps")
                for c in range(KC):
                    mm_tile(h_sb, ps, c)
                post_tile(t, ps)
    nc.finalize()
    return nc


def _np_mm_dtype(mm_dtype):
    return np.float16 if mm_dtype == F16 else np.float32


def _prep_inputs(h, W_, b, mm_dtype=F16):
    npdt = _np_mm_dtype(mm_dtype)
    # [k_in_chunk(p), chunk(c), expert(e)]: wtprep[p, c, e] = W[e, c*128 + p]
    wtprep = np.ascontiguousarray(
        W_.T.reshape(KC, 128, E).transpose(1, 0, 2).astype(npdt)
    )
    biasb = np.ascontiguousarray(np.broadcast_to(b, (128, E)))
    in_maps = []
    for core in range(N_CORES):
        hc = h[core * TOK:(core + 1) * TOK]
        # [tile, token_in_tile(j), chunk(c), k_in_chunk(p)] -> [tile, p, c, j]
        h4 = hc.reshape(TT, 128, KC, 128)
        htp = np.ascontiguousarray(h4.transpose(0, 3, 2, 1).astype(npdt))
        in_maps.append({"ht": htp, "wt": wtprep, "biasb": biasb})
    return in_maps


RISK_TAU = 1e-3  # local relative z-gap below which noise could flip order
_DBG = {}


def _epilogue(results, b, h_flat, W):
    idx_list, w_list, risk_list = [], [], []
    for r in results:
        pack = np.ascontiguousarray(r["o_pack"].reshape(-1, 37))
        z24 = pack[:, 0:24]                                       # [N, 24]
        i24 = pack[:, 24:36].view(np.uint16).astype(np.int32)     # local idx
        i24 = i24 + (np.arange(3, dtype=np.int32) * 256).repeat(8)[None, :]
        se = pack[:, 36:37]
        # merge the 3 per-third top-8 lists into a global top-16
        order = np.argsort(-z24, axis=-1, kind="stable")[:, :TOP16]
        z16 = np.take_along_axis(z24, order, axis=-1)
        idx16 = np.take_along_axis(i24, order, axis=-1)
        e16 = (z16 - se * b[idx16]).astype(np.float32)
        e12 = e16[:, :TOPK]
        denom = e12.sum(axis=-1, keepdims=True, dtype=np.float32) + np.float32(1e-20) * se
        w_list.append((np.float32(SCALE) * e12 / denom).astype(np.float32))
        idx_list.append(idx16[:, :TOPK].astype(np.int32))
        # flag tokens whose adjacent top-13 gaps are inside the noise band
        # (relative to the local z, not z1), or where one third contributed
        # >= 8 of the merged top-13 (its 9th candidate could be hidden)
        gaps = (z16[:, :TOPK + 1] - z16[:, 1:TOPK + 2]) / np.abs(z16[:, :TOPK + 1])
        third = order[:, :TOPK + 1] // 8
        crowd = (
            (third == 0).sum(-1) >= 8
        ) | ((third == 1).sum(-1) >= 8) | ((third == 2).sum(-1) >= 8)
        risk_list.append((gaps.min(axis=-1) < RISK_TAU) | crowd)
    topk_idx = np.concatenate(idx_list, axis=0)
    topk_w = np.concatenate(w_list, axis=0)

    # fp32-exact host recompute for at-risk tokens (mimics the reference op
    # sequence exactly in float32)
    risk = np.concatenate(risk_list, axis=0)
    _DBG["risk_frac"] = float(risk.mean())
    ridx = np.nonzero(risk)[0]
    if ridx.size:
        lg = h_flat[ridx] @ W.T.astype(np.float32)
        mx = lg.max(axis=-1, keepdims=True)
        ex = np.exp(lg - mx)
        s = ex / ex.sum(axis=-1, keepdims=True, dtype=np.float32)
        c = s + b
        ii = np.argsort(-c, axis=-1, kind="stable")[:, :TOPK]
        ww = np.take_along_axis(s, ii, axis=-1)
        ww = ww / (ww.sum(axis=-1, keepdims=True, dtype=np.float32) + np.float32(1e-20))
        topk_idx[ridx] = ii.astype(np.int32)
        topk_w[ridx] = (np.float32(SCALE) * ww).astype(np.float32)

    topk_idx = topk_idx.reshape(B, S, TOPK)
    topk_w = topk_w.reshape(B, S, TOPK).astype(np.float32)
    return topk_idx, topk_w


_NC_CACHE = {}


def run(hidden_states, W, e_score_correction_bias, trace=False, mm_dtype=F16):
    key = (str(mm_dtype),)
    if key not in _NC_CACHE:
        _NC_CACHE[key] = build_nc(mm_dtype)
    nc = _NC_CACHE[key]
    h = np.ascontiguousarray(np.asarray(hidden_states, dtype=np.float32)).reshape(-1, H)
    W_ = np.ascontiguousarray(np.asarray(W, dtype=np.float32))
    b = np.ascontiguousarray(np.asarray(e_score_correction_bias, dtype=np.float32))
    in_maps = _prep_inputs(h, W_, b, mm_dtype)
    res = run_bass_kernel_spmd(nc, in_maps, core_ids=list(range(N_CORES)), trace=trace)
    out = _epilogue(res.results, b, h, W_)
    if _DBG:
        print(f"risk fraction: {_DBG.get('risk_frac', -1):.4f}")
    return out, res


def kernel(hidden_states, W, e_score_correction_bias):
    out, _ = run(hidden_states, W, e_score_correction_bias, trace=False)
    return out
